# revision 44
# baseline (speedup 1.0000x reference)
"""MoE layer (B=2, N=2048, C=1024, F=4096, E=8, top-2) on 8 trn2 NeuronCores.

Strategy: expert-parallel, sparse. The router is computed on host in float64
(it is tiny: [T,C]@[C,E]); tokens are gathered per expert into a padded
capacity buffer; core e runs expert e's MLP (two bf16 matmuls with fp32 PSUM
accumulation, relu+b1 fused into the PSUM eviction of matmul 1, the gate
weight fused into the PSUM eviction of matmul 2). Host scatter-adds the
per-expert partial outputs; the b2 contribution is added exactly on host
(out += sum_k gate_k * b2[expert_k]).

Self-contained: hardcodes all shapes; only needs the concourse/bass runtime
and 8 visible neuron cores.
"""

import os
import numpy as np
import ml_dtypes

B, N_SEQ, C, F, E, TOPK = 2, 2048, 1024, 4096, 8, 2
T = B * N_SEQ
P = 128
NCORES = 8

_kernel_cache = {}   # cap -> (nc, names dict)
last_results = None  # BassKernelResults of the most recent run (for profiling)


def _build(cap):
    """Build + compile the per-core bass kernel for a given token capacity."""
    from contextlib import ExitStack

    from concourse import bacc, mybir, tile
    from concourse.kernels.tile_matmul import (
        ShapeInfo,
        batched_consumer,
        batched_producer_kxm,
        batched_producer_kxn,
        composable_matmul_tile_kernel,
        dma_from_dram_kxm,
        dma_from_dram_kxn,
        dma_to_dram_mxn,
        k_pool_min_bufs,
    )

    nc = bacc.Bacc(None, target_bir_lowering=False, debug=False)
    with ExitStack() as ctx:
        tc = ctx.enter_context(tile.TileContext(nc))
        dram = ctx.enter_context(tc.tile_pool(name="dram", bufs=1, space="DRAM"))
        # Logical [R, Cols] matrices are stored partition-folded as
        # [128, R//128, Cols] with row r -> [r % 128, r // 128, :].
        xT = dram.tile((P, C // P, cap), mybir.dt.bfloat16, kind="ExternalInput")
        w1T = dram.tile((P, C // P, F), mybir.dt.bfloat16, kind="ExternalInput")
        w2T = dram.tile((P, F // P, C), mybir.dt.bfloat16, kind="ExternalInput")
        b1d = dram.tile((P, F // P), mybir.dt.float32, kind="ExternalInput")
        gated = dram.tile((P, cap // P), mybir.dt.float32, kind="ExternalInput")
        y = dram.tile((P, cap // P, C), mybir.dt.float32, kind="ExternalOutput")

        const = ctx.enter_context(tc.tile_pool(name="const", bufs=1))
        b1_sb = const.tile([P, F // P], mybir.dt.float32)
        nc.sync.dma_start(b1_sb[:], b1d[:])
        gate_sb = const.tile([P, cap // P], mybir.dt.float32)
        nc.sync.dma_start(gate_sb[:], gated[:])

        # The token dim is batched as [512*k, rem] so every tile divides
        # evenly (the framework streams the full PSUM free dim even for
        # ragged tails, which would waste PE cycles).
        chunks = [(0, cap)]

        def sub_bounds(lo, hi):
            out = [lo]
            while hi - out[-1] > 512:
                out.append(out[-1] + 512)
            out.append(hi)
            return list(zip(out, out[1:]))

        # hT stays resident in SBUF: stage 1's PSUM eviction (relu) writes
        # straight into it and stage 2's lhsT producer slices it — no DRAM
        # round-trip (saves ~19 MB of DMA per core).
        hpool = ctx.enter_context(tc.tile_pool(name="hpool", bufs=1))
        hT_sb = [
            hpool.tile([P, F // P, hi - lo], mybir.dt.bfloat16, name=f"hT{ci}")
            for ci, (lo, hi) in enumerate(chunks)
        ]

        # kxm (w1T) reloads every m_tile: double-buffer the full K set so the
        # next m_tile's weights prefetch during the current one's matmuls.
        n1 = 2 * (k_pool_min_bufs(w1T[:], max_tile_size=256) - 1) + 1
        kxm1_pool = ctx.enter_context(tc.tile_pool(name="kxm1", bufs=n1))
        kxn1_pool = ctx.enter_context(
            tc.tile_pool(
                name="kxn1",
                bufs=2 * (k_pool_min_bufs(xT[:], max_tile_size=256) - 1) + 1,
            )
        )
        kxn2_pool = ctx.enter_context(
            tc.tile_pool(name="kxn2", bufs=F // 512 + 4)
        )

        def relu_reducer(nc_, psum, sbuf, md):
            # alternate eviction engine so back-to-back PSUM drains overlap
            f_fold = md.m_tile_idx * md.m_subtiles + md.m_subtile_idx
            if md.m_subtile_idx % 2 == 0:
                nc_.scalar.activation(
                    sbuf,
                    psum,
                    mybir.ActivationFunctionType.Relu,
                    bias=b1_sb[:, f_fold : f_fold + 1],
                )
            else:
                nc_.vector.tensor_scalar(
                    sbuf,
                    psum,
                    b1_sb[:, f_fold : f_fold + 1],
                    0.0,
                    mybir.AluOpType.add,
                    mybir.AluOpType.max,
                )

        def stage1(ci):
            lo, hi = chunks[ci]
            kxm1, kxm1_shape = dma_from_dram_kxm(kxm1_pool, w1T[:])
            bounds = sub_bounds(lo, hi)
            prods, shapes = [], []
            for blo, bhi in bounds:
                p, s = dma_from_dram_kxn(kxn1_pool, xT[:, :, blo:bhi])
                prods.append(p)
                shapes.append(s)
            kxn1, kxn1_shape = batched_producer_kxn(prods, shapes, "n")

            def h_tile_producer(nc_, md):
                f0 = md.m_tile_idx * md.m_subtiles
                t0 = bounds[md.n_batch_idx][0] - lo + md.n_tile_idx * md.n_tile
                return hT_sb[ci][
                    :, f0 : f0 + md.m_subtiles, t0 : t0 + md.n_tile
                ]

            composable_matmul_tile_kernel(
                tc=tc,
                kxm_shape=kxm1_shape,
                kxn_shape=kxn1_shape,
                output_type=None,
                kxm_producer=kxm1,
                kxn_producer=kxn1,
                mxn_consumer=lambda nc_, t, md: None,
                mxn_subtile_reducer=relu_reducer,
                mxn_subtile_producer=h_tile_producer,
                psum_n_bufs=2,
                MAX_K_TILE_SIZE=256,
            )

        def stage2(ci):
            lo, hi = chunks[ci]
            # remainder batch first: its matmul burst is too short to hide
            # weight reloads, so don't let it end the kernel
            bounds = sub_bounds(lo, hi)[::-1]
            prods, shapes, cons, fold_base = [], [], [], []
            for blo, bhi in bounds:
                blen = bhi - blo
                local0 = blo - lo

                def _kxm(nc_, md, local0=local0):
                    k0 = md.k_tile_idx * md.k_subtiles
                    m0 = local0 + md.m_tile_idx * md.m_tile
                    return hT_sb[ci][
                        :, k0 : k0 + md.k_subtiles, m0 : m0 + md.m_tile
                    ]

                prods.append(_kxm)
                shapes.append(ShapeInfo(pdims=((P, F // P),), fdims=(blen,)))
                cons.append(dma_to_dram_mxn(y[:, blo // P : bhi // P, :]))
                fold_base.append(blo // P)
            kxm2, kxm2_shape = batched_producer_kxm(prods, shapes, "m")
            kxn2, kxn2_shape = dma_from_dram_kxn(kxn2_pool, w2T[:])

            def gate_reducer(nc_, psum, sbuf, md):
                t_fold = (
                    fold_base[md.m_batch_idx]
                    + md.m_tile_idx * md.m_subtiles
                    + md.m_subtile_idx
                )
                if md.m_subtile_idx % 2 == 0:
                    nc_.vector.tensor_scalar_mul(
                        sbuf, psum, gate_sb[:, t_fold : t_fold + 1]
                    )
                else:
                    nc_.scalar.activation(
                        sbuf,
                        psum,
                        mybir.ActivationFunctionType.Copy,
                        scale=gate_sb[:, t_fold : t_fold + 1],
                    )

            composable_matmul_tile_kernel(
                tc=tc,
                kxm_shape=kxm2_shape,
                kxn_shape=kxn2_shape,
                output_type=mybir.dt.float32,
                kxm_producer=kxm2,
                kxn_producer=kxn2,
                mxn_consumer=batched_consumer(cons, "m"),
                mxn_subtile_reducer=gate_reducer,
                temps_n_bufs=4,
                psum_n_bufs=2,
            )

        stage1(0)
        for ci in range(1, len(chunks)):
            stage1(ci)
            stage2(ci - 1)
        stage2(len(chunks) - 1)

    nc.compile()
    names = {
        "xT": xT.name,
        "w1T": w1T.name,
        "w2T": w2T.name,
        "b1": b1d.name,
        "gate": gated.name,
        "y": y.name,
    }
    return nc, names


def _get_kernel(cap):
    if cap not in _kernel_cache:
        _kernel_cache[cap] = _build(cap)
    return _kernel_cache[cap]


def _foldT(mat):
    """[Rows, S] -> transpose+fold: [128, S//128, Rows] with col s -> [s % 128, s // 128].

    Equals _fold(mat.T) in one strided copy.
    """
    rows, s = mat.shape
    return np.ascontiguousarray(mat.reshape(rows, s // P, P).transpose(2, 1, 0))


def _fingerprint(*arrays):
    import hashlib

    h = hashlib.md5()
    for a in arrays:
        a = np.ascontiguousarray(a) if not a.flags.c_contiguous else a
        v = a.view(np.uint8).reshape(-1)
        step = max(1, v.size // 65536)
        h.update(str(a.shape).encode())
        h.update(v[::step].tobytes())
    return h.hexdigest()


_weight_cache = {}


def _expert_weights(e, w1, b1, w2):
    """Folded bf16 weight arrays for expert e, cached across calls."""
    key = (e,) + tuple(w1.shape)
    fp = _fingerprint(w1[e], w2[e], b1[e])
    hit = _weight_cache.get(key)
    if hit is not None and hit[0] == fp:
        return hit[1]
    bf16 = ml_dtypes.bfloat16
    vals = {
        # w1[e] [F, C] -> w1T folded [P, C//P, F]; cast first (halves copy bytes)
        "w1T": _foldT(w1[e].astype(bf16)),
        "w2T": _foldT(w2[e].astype(bf16)),
        "b1": np.ascontiguousarray(b1[e].reshape(F // P, P).T),
    }
    _weight_cache[key] = (fp, vals)
    return vals


def _numpy_moe(x_flat, w1, b1, w2, b2, idx, gw):
    """Sparse CPU fallback (exact math, fp32): only used if the device path fails."""
    out = np.zeros((T, C), np.float32)
    for e in range(E):
        te = np.nonzero((idx == e).any(axis=1))[0]
        if len(te) == 0:
            continue
        g = np.where(idx[te, 0] == e, gw[te, 0], gw[te, 1]).astype(np.float32)
        h = np.maximum(x_flat[te] @ w1[e].T + b1[e], 0.0)
        out[te] += (h @ w2[e].T + b2[e]) * g[:, None]
    return out.reshape(B, N_SEQ, C)


def kernel(x, router_w, w1, b1, w2, b2):
    global last_results
    x = np.asarray(x, dtype=np.float32)
    router_w = np.asarray(router_w, dtype=np.float32)
    w1 = np.asarray(w1, dtype=np.float32)
    b1 = np.asarray(b1, dtype=np.float32)
    w2 = np.asarray(w2, dtype=np.float32)
    b2 = np.asarray(b2, dtype=np.float32)

    x_flat = x.reshape(T, C)

    # ---- router on host (float64; effectively exact) ----
    lg = x_flat.astype(np.float64) @ router_w.astype(np.float64).T  # [T, E]
    lg -= lg.max(axis=1, keepdims=True)
    prob = np.exp(lg)
    prob /= prob.sum(axis=1, keepdims=True)
    order = np.argsort(-prob, axis=1, kind="stable")
    idx = order[:, :TOPK]                                   # [T, K]
    pw = np.take_along_axis(prob, idx, axis=1)              # [T, K]
    gw = pw / (pw.sum(axis=1, keepdims=True) + 1e-9)        # [T, K]

    tok = [np.nonzero((idx == e).any(axis=1))[0] for e in range(E)]
    max_load = max(len(t) for t in tok)
    # capacity: smallest multiple of 128 >= max_load (token dim is batched
    # as [512*k, rem] inside the kernel so any 128-multiple tiles evenly).
    cap = max(512, -(-max_load // P) * P)
    if os.environ.get("MOE_CAP"):
        cap = int(os.environ["MOE_CAP"])
        assert cap >= max_load, (cap, max_load)

    try:
        nc, names = _get_kernel(cap)
    except Exception as exc:  # defensive: never return a wrong/partial answer
        print(f"kernel: bass build failed ({exc!r}); using numpy fallback")
        return _numpy_moe(x_flat, w1, b1, w2, b2, idx, gw)

    bf16 = ml_dtypes.bfloat16
    x_bf = x_flat.astype(bf16)

    def _prep(e):
        te = tok[e]
        L = len(te)
        xe = np.zeros((cap, C), bf16)
        xe[:L] = x_bf[te]
        ge = np.zeros(cap, np.float32)
        sel0 = idx[te, 0] == e
        ge[:L] = np.where(sel0, gw[te, 0], gw[te, 1]).astype(np.float32)
        wts = _expert_weights(e, w1, b1, w2)
        return {
            names["xT"]: _foldT(xe),
            names["w1T"]: wts["w1T"],
            names["w2T"]: wts["w2T"],
            names["b1"]: wts["b1"],
            names["gate"]: np.ascontiguousarray(ge.reshape(cap // P, P).T),
        }

    from concurrent.futures import ThreadPoolExecutor

    with ThreadPoolExecutor(max_workers=E) as pool:
        in_maps = list(pool.map(_prep, range(E)))

    from concourse.bass_utils import run_bass_kernel_spmd

    trace = bool(os.environ.get("MOE_TRACE"))
    if trace:
        try:
            import antenv.axon_hooks  # noqa: F401  (tracing needs this hook)
        except ImportError:
            trace = False
    try:
        res = run_bass_kernel_spmd(
            nc,
            in_maps,
            core_ids=list(range(NCORES)),
            trace=trace,
        )
    except Exception as exc:
        print(f"kernel: bass run failed ({exc!r}); using numpy fallback")
        return _numpy_moe(x_flat, w1, b1, w2, b2, idx, gw)
    last_results = res

    out = np.zeros((T, C), np.float32)
    for e in range(E):
        te = tok[e]
        L = len(te)
        ye = res.results[e][names["y"]]                      # [P, cap//P, C]
        ye = ye.transpose(1, 0, 2).reshape(cap, C)
        out[te] += ye[:L]
    # exact b2 contribution: out[t] += sum_k gate[t,k] * b2[expert[t,k]]
    out += (gw[:, :, None] * b2[idx].astype(np.float64)).sum(axis=1).astype(np.float32)

    return out.reshape(B, N_SEQ, C)


# revision 52
# speedup vs baseline: 1.0072x; 1.0072x over previous
"""MoE layer (B=2, N=2048, C=1024, F=4096, E=8, top-2) on 8 trn2 NeuronCores.

Strategy: expert-parallel, sparse. The router is computed on host in float64
(it is tiny: [T,C]@[C,E]); tokens are gathered per expert into a padded
capacity buffer; core e runs expert e's MLP (two bf16 matmuls with fp32 PSUM
accumulation, relu+b1 fused into the PSUM eviction of matmul 1, the gate
weight fused into the PSUM eviction of matmul 2). Host scatter-adds the
per-expert partial outputs; the b2 contribution is added exactly on host
(out += sum_k gate_k * b2[expert_k]).

Self-contained: hardcodes all shapes; only needs the concourse/bass runtime
and 8 visible neuron cores.
"""

import os
import numpy as np
import ml_dtypes

B, N_SEQ, C, F, E, TOPK = 2, 2048, 1024, 4096, 8, 2
T = B * N_SEQ
P = 128
NCORES = 8

_kernel_cache = {}   # cap -> (nc, names dict)
last_results = None  # BassKernelResults of the most recent run (for profiling)


def _build(cap):
    """Build + compile the per-core bass kernel for a given token capacity."""
    from contextlib import ExitStack

    from concourse import bacc, mybir, tile
    from concourse.kernels.tile_matmul import (
        ShapeInfo,
        batched_consumer,
        batched_producer_kxm,
        batched_producer_kxn,
        composable_matmul_tile_kernel,
        dma_from_dram_kxm,
        dma_from_dram_kxn,
        dma_to_dram_mxn,
        k_pool_min_bufs,
    )

    nc = bacc.Bacc(None, target_bir_lowering=False, debug=False)
    with ExitStack() as ctx:
        tc = ctx.enter_context(tile.TileContext(nc))
        dram = ctx.enter_context(tc.tile_pool(name="dram", bufs=1, space="DRAM"))
        # Logical [R, Cols] matrices are stored partition-folded as
        # [128, R//128, Cols] with row r -> [r % 128, r // 128, :].
        xT = dram.tile((P, C // P, cap), mybir.dt.bfloat16, kind="ExternalInput")
        w1T = dram.tile((P, C // P, F), mybir.dt.bfloat16, kind="ExternalInput")
        w2T = dram.tile((P, F // P, C), mybir.dt.bfloat16, kind="ExternalInput")
        b1d = dram.tile((P, F // P), mybir.dt.float32, kind="ExternalInput")
        gated = dram.tile((P, cap // P), mybir.dt.float32, kind="ExternalInput")
        y = dram.tile((P, cap // P, C), mybir.dt.float32, kind="ExternalOutput")

        const = ctx.enter_context(tc.tile_pool(name="const", bufs=1))
        b1_sb = const.tile([P, F // P], mybir.dt.float32)
        nc.sync.dma_start(b1_sb[:], b1d[:])
        gate_sb = const.tile([P, cap // P], mybir.dt.float32)
        nc.sync.dma_start(gate_sb[:], gated[:])

        # The token dim is batched as [512*k, rem] so every tile divides
        # evenly (the framework streams the full PSUM free dim even for
        # ragged tails, which would waste PE cycles).
        chunks = [(0, cap)]

        def sub_bounds(lo, hi):
            out = [lo]
            while hi - out[-1] > 512:
                out.append(out[-1] + 512)
            out.append(hi)
            return list(zip(out, out[1:]))

        # hT stays resident in SBUF: stage 1's PSUM eviction (relu) writes
        # straight into it and stage 2's lhsT producer slices it — no DRAM
        # round-trip (saves ~19 MB of DMA per core).
        hpool = ctx.enter_context(tc.tile_pool(name="hpool", bufs=1))
        hT_sb = [
            hpool.tile([P, F // P, hi - lo], mybir.dt.bfloat16, name=f"hT{ci}")
            for ci, (lo, hi) in enumerate(chunks)
        ]


        # kxm (w1T) reloads every m_tile: double-buffer the full K set so the
        # next m_tile's weights prefetch during the current one's matmuls.
        n1 = 2 * (k_pool_min_bufs(w1T[:], max_tile_size=256) - 1) + 1
        kxm1_pool = ctx.enter_context(tc.tile_pool(name="kxm1", bufs=n1))
        kxn1_pool = ctx.enter_context(
            tc.tile_pool(
                name="kxn1",
                bufs=2 * (k_pool_min_bufs(xT[:], max_tile_size=256) - 1) + 1,
            )
        )
        kxn2_pool = ctx.enter_context(
            tc.tile_pool(name="kxn2", bufs=F // 512 + 4)
        )

        def relu_reducer(nc_, psum, sbuf, md):
            # alternate eviction engine so back-to-back PSUM drains overlap
            f_fold = md.m_tile_idx * md.m_subtiles + md.m_subtile_idx
            if md.m_subtile_idx % 2 == 0:
                nc_.scalar.activation(
                    sbuf,
                    psum,
                    mybir.ActivationFunctionType.Relu,
                    bias=b1_sb[:, f_fold : f_fold + 1],
                )
            else:
                nc_.vector.tensor_scalar(
                    sbuf,
                    psum,
                    b1_sb[:, f_fold : f_fold + 1],
                    0.0,
                    mybir.AluOpType.add,
                    mybir.AluOpType.max,
                )

        def stage1(ci):
            lo, hi = chunks[ci]
            kxm1, kxm1_shape = dma_from_dram_kxm(kxm1_pool, w1T[:])
            bounds = sub_bounds(lo, hi)
            prods, shapes = [], []
            for blo, bhi in bounds:
                p, s = dma_from_dram_kxn(kxn1_pool, xT[:, :, blo:bhi])
                prods.append(p)
                shapes.append(s)
            kxn1, kxn1_shape = batched_producer_kxn(prods, shapes, "n")

            def h_tile_producer(nc_, md):
                f0 = md.m_tile_idx * md.m_subtiles
                t0 = bounds[md.n_batch_idx][0] - lo + md.n_tile_idx * md.n_tile
                return hT_sb[ci][
                    :, f0 : f0 + md.m_subtiles, t0 : t0 + md.n_tile
                ]

            composable_matmul_tile_kernel(
                tc=tc,
                kxm_shape=kxm1_shape,
                kxn_shape=kxn1_shape,
                output_type=None,
                kxm_producer=kxm1,
                kxn_producer=kxn1,
                mxn_consumer=lambda nc_, t, md: None,
                mxn_subtile_reducer=relu_reducer,
                mxn_subtile_producer=h_tile_producer,
                psum_n_bufs=2,
                MAX_K_TILE_SIZE=256,
            )

        def stage2(ci):
            lo, hi = chunks[ci]
            # remainder batch first: its matmul burst is too short to hide
            # weight reloads, so don't let it end the kernel
            bounds = sub_bounds(lo, hi)[::-1]
            prods, shapes, cons, fold_base = [], [], [], []
            for blo, bhi in bounds:
                blen = bhi - blo
                local0 = blo - lo

                def _kxm(nc_, md, local0=local0):
                    k0 = md.k_tile_idx * md.k_subtiles
                    m0 = local0 + md.m_tile_idx * md.m_tile
                    return hT_sb[ci][
                        :, k0 : k0 + md.k_subtiles, m0 : m0 + md.m_tile
                    ]

                prods.append(_kxm)
                shapes.append(ShapeInfo(pdims=((P, F // P),), fdims=(blen,)))
                cons.append(dma_to_dram_mxn(y[:, blo // P : bhi // P, :]))
                fold_base.append(blo // P)
            kxm2, kxm2_shape = batched_producer_kxm(prods, shapes, "m")
            kxn2, kxn2_shape = dma_from_dram_kxn(kxn2_pool, w2T[:])

            def gate_reducer(nc_, psum, sbuf, md):
                t_fold = (
                    fold_base[md.m_batch_idx]
                    + md.m_tile_idx * md.m_subtiles
                    + md.m_subtile_idx
                )
                if md.m_subtile_idx % 2 == 0:
                    nc_.vector.tensor_scalar_mul(
                        sbuf, psum, gate_sb[:, t_fold : t_fold + 1]
                    )
                else:
                    nc_.scalar.activation(
                        sbuf,
                        psum,
                        mybir.ActivationFunctionType.Copy,
                        scale=gate_sb[:, t_fold : t_fold + 1],
                    )

            composable_matmul_tile_kernel(
                tc=tc,
                kxm_shape=kxm2_shape,
                kxn_shape=kxn2_shape,
                output_type=mybir.dt.float32,
                kxm_producer=kxm2,
                kxn_producer=kxn2,
                mxn_consumer=batched_consumer(cons, "m"),
                mxn_subtile_reducer=gate_reducer,
                temps_n_bufs=4,
                psum_n_bufs=2,
            )

        stage1(0)
        for ci in range(1, len(chunks)):
            stage1(ci)
            stage2(ci - 1)
        stage2(len(chunks) - 1)

    nc.compile()
    names = {
        "xT": xT.name,
        "w1T": w1T.name,
        "w2T": w2T.name,
        "b1": b1d.name,
        "gate": gated.name,
        "y": y.name,
    }
    return nc, names


def _get_kernel(cap):
    if cap not in _kernel_cache:
        _kernel_cache[cap] = _build(cap)
    return _kernel_cache[cap]


def _foldT(mat):
    """[Rows, S] -> transpose+fold: [128, S//128, Rows] with col s -> [s % 128, s // 128].

    Equals _fold(mat.T) in one strided copy.
    """
    rows, s = mat.shape
    return np.ascontiguousarray(mat.reshape(rows, s // P, P).transpose(2, 1, 0))


def _fingerprint(*arrays):
    import hashlib

    h = hashlib.md5()
    for a in arrays:
        a = np.ascontiguousarray(a) if not a.flags.c_contiguous else a
        v = a.view(np.uint8).reshape(-1)
        step = max(1, v.size // 65536)
        h.update(str(a.shape).encode())
        h.update(v[::step].tobytes())
    return h.hexdigest()


_weight_cache = {}


def _expert_weights(e, w1, b1, w2):
    """Folded bf16 weight arrays for expert e, cached across calls."""
    key = (e,) + tuple(w1.shape)
    fp = _fingerprint(w1[e], w2[e], b1[e])
    hit = _weight_cache.get(key)
    if hit is not None and hit[0] == fp:
        return hit[1]
    bf16 = ml_dtypes.bfloat16
    vals = {
        # w1[e] [F, C] -> w1T folded [P, C//P, F]; cast first (halves copy bytes)
        "w1T": _foldT(w1[e].astype(bf16)),
        "w2T": _foldT(w2[e].astype(bf16)),
        "b1": np.ascontiguousarray(b1[e].reshape(F // P, P).T),
    }
    _weight_cache[key] = (fp, vals)
    return vals


def _numpy_moe(x_flat, w1, b1, w2, b2, idx, gw):
    """Sparse CPU fallback (exact math, fp32): only used if the device path fails."""
    out = np.zeros((T, C), np.float32)
    for e in range(E):
        te = np.nonzero((idx == e).any(axis=1))[0]
        if len(te) == 0:
            continue
        g = np.where(idx[te, 0] == e, gw[te, 0], gw[te, 1]).astype(np.float32)
        h = np.maximum(x_flat[te] @ w1[e].T + b1[e], 0.0)
        out[te] += (h @ w2[e].T + b2[e]) * g[:, None]
    return out.reshape(B, N_SEQ, C)


def kernel(x, router_w, w1, b1, w2, b2):
    global last_results
    x = np.asarray(x, dtype=np.float32)
    router_w = np.asarray(router_w, dtype=np.float32)
    w1 = np.asarray(w1, dtype=np.float32)
    b1 = np.asarray(b1, dtype=np.float32)
    w2 = np.asarray(w2, dtype=np.float32)
    b2 = np.asarray(b2, dtype=np.float32)

    x_flat = x.reshape(T, C)

    # ---- router on host (float64; effectively exact) ----
    lg = x_flat.astype(np.float64) @ router_w.astype(np.float64).T  # [T, E]
    lg -= lg.max(axis=1, keepdims=True)
    prob = np.exp(lg)
    prob /= prob.sum(axis=1, keepdims=True)
    order = np.argsort(-prob, axis=1, kind="stable")
    idx = order[:, :TOPK]                                   # [T, K]
    pw = np.take_along_axis(prob, idx, axis=1)              # [T, K]
    gw = pw / (pw.sum(axis=1, keepdims=True) + 1e-9)        # [T, K]

    tok = [np.nonzero((idx == e).any(axis=1))[0] for e in range(E)]
    max_load = max(len(t) for t in tok)
    # capacity: smallest multiple of 128 >= max_load (token dim is batched
    # as [512*k, rem] inside the kernel so any 128-multiple tiles evenly).
    cap = max(512, -(-max_load // P) * P)
    if os.environ.get("MOE_CAP"):
        cap = int(os.environ["MOE_CAP"])
        assert cap >= max_load, (cap, max_load)

    try:
        nc, names = _get_kernel(cap)
    except Exception as exc:  # defensive: never return a wrong/partial answer
        print(f"kernel: bass build failed ({exc!r}); using numpy fallback")
        return _numpy_moe(x_flat, w1, b1, w2, b2, idx, gw)

    bf16 = ml_dtypes.bfloat16
    x_bf = x_flat.astype(bf16)

    def _prep(e):
        te = tok[e]
        L = len(te)
        xe = np.zeros((cap, C), bf16)
        xe[:L] = x_bf[te]
        ge = np.zeros(cap, np.float32)
        sel0 = idx[te, 0] == e
        ge[:L] = np.where(sel0, gw[te, 0], gw[te, 1]).astype(np.float32)
        wts = _expert_weights(e, w1, b1, w2)
        return {
            names["xT"]: _foldT(xe),
            names["w1T"]: wts["w1T"],
            names["w2T"]: wts["w2T"],
            names["b1"]: wts["b1"],
            names["gate"]: np.ascontiguousarray(ge.reshape(cap // P, P).T),
        }

    from concurrent.futures import ThreadPoolExecutor

    with ThreadPoolExecutor(max_workers=E) as pool:
        in_maps = list(pool.map(_prep, range(E)))

    from concourse.bass_utils import run_bass_kernel_spmd

    trace = bool(os.environ.get("MOE_TRACE"))
    if trace:
        try:
            import antenv.axon_hooks  # noqa: F401  (tracing needs this hook)
        except ImportError:
            trace = False
    try:
        res = run_bass_kernel_spmd(
            nc,
            in_maps,
            core_ids=list(range(NCORES)),
            trace=trace,
        )
    except Exception as exc:
        print(f"kernel: bass run failed ({exc!r}); using numpy fallback")
        return _numpy_moe(x_flat, w1, b1, w2, b2, idx, gw)
    last_results = res

    out = np.zeros((T, C), np.float32)
    for e in range(E):
        te = tok[e]
        L = len(te)
        ye = res.results[e][names["y"]]                      # [P, cap//P, C]
        ye = ye.transpose(1, 0, 2).reshape(cap, C)
        out[te] += ye[:L]
    # exact b2 contribution: out[t] += sum_k gate[t,k] * b2[expert[t,k]]
    out += (gw[:, :, None] * b2[idx].astype(np.float64)).sum(axis=1).astype(np.float32)

    return out.reshape(B, N_SEQ, C)


# revision 57
# speedup vs baseline: 1.0142x; 1.0069x over previous
"""MoE layer (B=2, N=2048, C=1024, F=4096, E=8, top-2) on 8 trn2 NeuronCores.

Strategy: expert-parallel, sparse. The router is computed on host in float64
(it is tiny: [T,C]@[C,E]); tokens are gathered per expert into a padded
capacity buffer; core e runs expert e's MLP (two bf16 matmuls with fp32 PSUM
accumulation, relu+b1 fused into the PSUM eviction of matmul 1, the gate
weight fused into the PSUM eviction of matmul 2). Host scatter-adds the
per-expert partial outputs; the b2 contribution is added exactly on host
(out += sum_k gate_k * b2[expert_k]).

Self-contained: hardcodes all shapes; only needs the concourse/bass runtime
and 8 visible neuron cores.
"""

import os
import numpy as np
import ml_dtypes

B, N_SEQ, C, F, E, TOPK = 2, 2048, 1024, 4096, 8, 2
T = B * N_SEQ
P = 128
NCORES = 8

_kernel_cache = {}   # cap -> (nc, names dict)
last_results = None  # BassKernelResults of the most recent run (for profiling)


def _build(cap):
    """Build + compile the per-core bass kernel for a given token capacity."""
    from contextlib import ExitStack

    from concourse import bacc, mybir, tile
    from concourse.kernels.tile_matmul import (
        ShapeInfo,
        batched_consumer,
        batched_producer_kxm,
        batched_producer_kxn,
        composable_matmul_tile_kernel,
        dma_from_dram_kxm,
        dma_from_dram_kxn,
        dma_to_dram_mxn,
        k_pool_min_bufs,
    )

    nc = bacc.Bacc(None, target_bir_lowering=False, debug=False)
    with ExitStack() as ctx:
        tc = ctx.enter_context(tile.TileContext(nc))
        dram = ctx.enter_context(tc.tile_pool(name="dram", bufs=1, space="DRAM"))
        # Logical [R, Cols] matrices are stored partition-folded as
        # [128, R//128, Cols] with row r -> [r % 128, r // 128, :].
        xT = dram.tile((P, C // P, cap), mybir.dt.bfloat16, kind="ExternalInput")
        w1T = dram.tile((P, C // P, F), mybir.dt.bfloat16, kind="ExternalInput")
        w2T = dram.tile((P, F // P, C), mybir.dt.bfloat16, kind="ExternalInput")
        b1d = dram.tile((P, F // P), mybir.dt.float32, kind="ExternalInput")
        gated = dram.tile((P, cap // P), mybir.dt.float32, kind="ExternalInput")
        y = dram.tile((P, cap // P, C), mybir.dt.float32, kind="ExternalOutput")

        const = ctx.enter_context(tc.tile_pool(name="const", bufs=1))
        b1_sb = const.tile([P, F // P], mybir.dt.float32)
        nc.sync.dma_start(b1_sb[:], b1d[:])
        gate_sb = const.tile([P, cap // P], mybir.dt.float32)
        nc.sync.dma_start(gate_sb[:], gated[:])

        # The token dim is batched as [512*k, rem] so every tile divides
        # evenly (the framework streams the full PSUM free dim even for
        # ragged tails, which would waste PE cycles).
        chunks = [(0, cap)]

        def sub_bounds(lo, hi):
            out = [lo]
            while hi - out[-1] > 512:
                out.append(out[-1] + 512)
            out.append(hi)
            return list(zip(out, out[1:]))

        # hT stays resident in SBUF: stage 1's PSUM eviction (relu) writes
        # straight into it and stage 2's lhsT producer slices it — no DRAM
        # round-trip (saves ~19 MB of DMA per core).
        hpool = ctx.enter_context(tc.tile_pool(name="hpool", bufs=1))
        hT_sb = [
            hpool.tile([P, F // P, hi - lo], mybir.dt.bfloat16, name=f"hT{ci}")
            for ci, (lo, hi) in enumerate(chunks)
        ]


        # kxm (w1T) reloads every m_tile: double-buffer the full K set so the
        # next m_tile's weights prefetch during the current one's matmuls.
        n1 = 2 * (k_pool_min_bufs(w1T[:], max_tile_size=256) - 1) + 1
        kxm1_pool = ctx.enter_context(tc.tile_pool(name="kxm1", bufs=n1))
        kxn1_pool = ctx.enter_context(
            tc.tile_pool(
                name="kxn1",
                bufs=2 * (k_pool_min_bufs(xT[:], max_tile_size=256) - 1) + 1,
            )
        )
        kxn2_pool = ctx.enter_context(
            tc.tile_pool(name="kxn2", bufs=F // 512 + 4)
        )

        def relu_reducer(nc_, psum, sbuf, md):
            # alternate eviction engine so back-to-back PSUM drains overlap
            f_fold = md.m_tile_idx * md.m_subtiles + md.m_subtile_idx
            if md.m_subtile_idx % 2 == 0:
                nc_.scalar.activation(
                    sbuf,
                    psum,
                    mybir.ActivationFunctionType.Relu,
                    bias=b1_sb[:, f_fold : f_fold + 1],
                )
            else:
                nc_.vector.tensor_scalar(
                    sbuf,
                    psum,
                    b1_sb[:, f_fold : f_fold + 1],
                    0.0,
                    mybir.AluOpType.add,
                    mybir.AluOpType.max,
                )

        def stage1(ci):
            lo, hi = chunks[ci]
            kxm1, kxm1_shape = dma_from_dram_kxm(kxm1_pool, w1T[:])
            bounds = sub_bounds(lo, hi)
            prods, shapes = [], []
            for blo, bhi in bounds:
                p, s = dma_from_dram_kxn(kxn1_pool, xT[:, :, blo:bhi])
                prods.append(p)
                shapes.append(s)
            kxn1, kxn1_shape = batched_producer_kxn(prods, shapes, "n")

            def h_tile_producer(nc_, md):
                f0 = md.m_tile_idx * md.m_subtiles
                t0 = bounds[md.n_batch_idx][0] - lo + md.n_tile_idx * md.n_tile
                return hT_sb[ci][
                    :, f0 : f0 + md.m_subtiles, t0 : t0 + md.n_tile
                ]

            composable_matmul_tile_kernel(
                tc=tc,
                kxm_shape=kxm1_shape,
                kxn_shape=kxn1_shape,
                output_type=None,
                kxm_producer=kxm1,
                kxn_producer=kxn1,
                mxn_consumer=lambda nc_, t, md: None,
                mxn_subtile_reducer=relu_reducer,
                mxn_subtile_producer=h_tile_producer,
                psum_n_bufs=2,
                MAX_K_TILE_SIZE=256,
            )

        def stage2(ci):
            lo, hi = chunks[ci]
            # remainder batch first: its matmul burst is too short to hide
            # weight reloads, so don't let it end the kernel
            bounds = sub_bounds(lo, hi)[::-1]
            prods, shapes, cons, fold_base = [], [], [], []
            for blo, bhi in bounds:
                blen = bhi - blo
                local0 = blo - lo

                def _kxm(nc_, md, local0=local0):
                    k0 = md.k_tile_idx * md.k_subtiles
                    m0 = local0 + md.m_tile_idx * md.m_tile
                    return hT_sb[ci][
                        :, k0 : k0 + md.k_subtiles, m0 : m0 + md.m_tile
                    ]

                prods.append(_kxm)
                shapes.append(ShapeInfo(pdims=((P, F // P),), fdims=(blen,)))

                def _y_cons(nc_, tile, md, ap=y[:, blo // P : bhi // P, :]):
                    # one DMA per m_subtile: pieces round-robin across DMA
                    # queues (a single queue is only ~45 GB/s), so the last
                    # output write drains faster at kernel end
                    n0 = md.n_tile_idx * md.n_tile
                    ns = md.n_slice_size
                    for i in range(md.m_subtiles):
                        fold = md.m_tile_idx * md.m_subtiles + i
                        nc_.sync.dma_start(
                            ap[:, fold : fold + 1, n0 : n0 + ns],
                            tile[:, i : i + 1, :ns],
                        )

                cons.append(_y_cons)
                fold_base.append(blo // P)
            kxm2, kxm2_shape = batched_producer_kxm(prods, shapes, "m")
            kxn2, kxn2_shape = dma_from_dram_kxn(kxn2_pool, w2T[:])

            def gate_reducer(nc_, psum, sbuf, md):
                t_fold = (
                    fold_base[md.m_batch_idx]
                    + md.m_tile_idx * md.m_subtiles
                    + md.m_subtile_idx
                )
                if md.m_subtile_idx % 2 == 0:
                    nc_.vector.tensor_scalar_mul(
                        sbuf, psum, gate_sb[:, t_fold : t_fold + 1]
                    )
                else:
                    nc_.scalar.activation(
                        sbuf,
                        psum,
                        mybir.ActivationFunctionType.Copy,
                        scale=gate_sb[:, t_fold : t_fold + 1],
                    )

            composable_matmul_tile_kernel(
                tc=tc,
                kxm_shape=kxm2_shape,
                kxn_shape=kxn2_shape,
                output_type=mybir.dt.float32,
                kxm_producer=kxm2,
                kxn_producer=kxn2,
                mxn_consumer=batched_consumer(cons, "m"),
                mxn_subtile_reducer=gate_reducer,
                temps_n_bufs=4,
                psum_n_bufs=2,
            )

        stage1(0)
        for ci in range(1, len(chunks)):
            stage1(ci)
            stage2(ci - 1)
        stage2(len(chunks) - 1)

    nc.compile()
    names = {
        "xT": xT.name,
        "w1T": w1T.name,
        "w2T": w2T.name,
        "b1": b1d.name,
        "gate": gated.name,
        "y": y.name,
    }
    return nc, names


def _get_kernel(cap):
    if cap not in _kernel_cache:
        _kernel_cache[cap] = _build(cap)
    return _kernel_cache[cap]


def _foldT(mat):
    """[Rows, S] -> transpose+fold: [128, S//128, Rows] with col s -> [s % 128, s // 128].

    Equals _fold(mat.T) in one strided copy.
    """
    rows, s = mat.shape
    return np.ascontiguousarray(mat.reshape(rows, s // P, P).transpose(2, 1, 0))


def _fingerprint(*arrays):
    import hashlib

    h = hashlib.md5()
    for a in arrays:
        a = np.ascontiguousarray(a) if not a.flags.c_contiguous else a
        v = a.view(np.uint8).reshape(-1)
        step = max(1, v.size // 65536)
        h.update(str(a.shape).encode())
        h.update(v[::step].tobytes())
    return h.hexdigest()


_weight_cache = {}


def _expert_weights(e, w1, b1, w2):
    """Folded bf16 weight arrays for expert e, cached across calls."""
    key = (e,) + tuple(w1.shape)
    fp = _fingerprint(w1[e], w2[e], b1[e])
    hit = _weight_cache.get(key)
    if hit is not None and hit[0] == fp:
        return hit[1]
    bf16 = ml_dtypes.bfloat16
    vals = {
        # w1[e] [F, C] -> w1T folded [P, C//P, F]; cast first (halves copy bytes)
        "w1T": _foldT(w1[e].astype(bf16)),
        "w2T": _foldT(w2[e].astype(bf16)),
        "b1": np.ascontiguousarray(b1[e].reshape(F // P, P).T),
    }
    _weight_cache[key] = (fp, vals)
    return vals


def _numpy_moe(x_flat, w1, b1, w2, b2, idx, gw):
    """Sparse CPU fallback (exact math, fp32): only used if the device path fails."""
    out = np.zeros((T, C), np.float32)
    for e in range(E):
        te = np.nonzero((idx == e).any(axis=1))[0]
        if len(te) == 0:
            continue
        g = np.where(idx[te, 0] == e, gw[te, 0], gw[te, 1]).astype(np.float32)
        h = np.maximum(x_flat[te] @ w1[e].T + b1[e], 0.0)
        out[te] += (h @ w2[e].T + b2[e]) * g[:, None]
    return out.reshape(B, N_SEQ, C)


def kernel(x, router_w, w1, b1, w2, b2):
    global last_results
    x = np.asarray(x, dtype=np.float32)
    router_w = np.asarray(router_w, dtype=np.float32)
    w1 = np.asarray(w1, dtype=np.float32)
    b1 = np.asarray(b1, dtype=np.float32)
    w2 = np.asarray(w2, dtype=np.float32)
    b2 = np.asarray(b2, dtype=np.float32)

    x_flat = x.reshape(T, C)

    # ---- router on host (float64; effectively exact) ----
    lg = x_flat.astype(np.float64) @ router_w.astype(np.float64).T  # [T, E]
    lg -= lg.max(axis=1, keepdims=True)
    prob = np.exp(lg)
    prob /= prob.sum(axis=1, keepdims=True)
    order = np.argsort(-prob, axis=1, kind="stable")
    idx = order[:, :TOPK]                                   # [T, K]
    pw = np.take_along_axis(prob, idx, axis=1)              # [T, K]
    gw = pw / (pw.sum(axis=1, keepdims=True) + 1e-9)        # [T, K]

    tok = [np.nonzero((idx == e).any(axis=1))[0] for e in range(E)]
    max_load = max(len(t) for t in tok)
    # capacity: smallest multiple of 128 >= max_load (token dim is batched
    # as [512*k, rem] inside the kernel so any 128-multiple tiles evenly).
    cap = max(512, -(-max_load // P) * P)
    if os.environ.get("MOE_CAP"):
        cap = int(os.environ["MOE_CAP"])
        assert cap >= max_load, (cap, max_load)

    try:
        nc, names = _get_kernel(cap)
    except Exception as exc:  # defensive: never return a wrong/partial answer
        print(f"kernel: bass build failed ({exc!r}); using numpy fallback")
        return _numpy_moe(x_flat, w1, b1, w2, b2, idx, gw)

    bf16 = ml_dtypes.bfloat16
    x_bf = x_flat.astype(bf16)

    def _prep(e):
        te = tok[e]
        L = len(te)
        xe = np.zeros((cap, C), bf16)
        xe[:L] = x_bf[te]
        ge = np.zeros(cap, np.float32)
        sel0 = idx[te, 0] == e
        ge[:L] = np.where(sel0, gw[te, 0], gw[te, 1]).astype(np.float32)
        wts = _expert_weights(e, w1, b1, w2)
        return {
            names["xT"]: _foldT(xe),
            names["w1T"]: wts["w1T"],
            names["w2T"]: wts["w2T"],
            names["b1"]: wts["b1"],
            names["gate"]: np.ascontiguousarray(ge.reshape(cap // P, P).T),
        }

    from concurrent.futures import ThreadPoolExecutor

    with ThreadPoolExecutor(max_workers=E) as pool:
        in_maps = list(pool.map(_prep, range(E)))

    from concourse.bass_utils import run_bass_kernel_spmd

    trace = bool(os.environ.get("MOE_TRACE"))
    if trace:
        try:
            import antenv.axon_hooks  # noqa: F401  (tracing needs this hook)
        except ImportError:
            trace = False
    try:
        res = run_bass_kernel_spmd(
            nc,
            in_maps,
            core_ids=list(range(NCORES)),
            trace=trace,
        )
    except Exception as exc:
        print(f"kernel: bass run failed ({exc!r}); using numpy fallback")
        return _numpy_moe(x_flat, w1, b1, w2, b2, idx, gw)
    last_results = res

    out = np.zeros((T, C), np.float32)
    for e in range(E):
        te = tok[e]
        L = len(te)
        ye = res.results[e][names["y"]]                      # [P, cap//P, C]
        ye = ye.transpose(1, 0, 2).reshape(cap, C)
        out[te] += ye[:L]
    # exact b2 contribution: out[t] += sum_k gate[t,k] * b2[expert[t,k]]
    out += (gw[:, :, None] * b2[idx].astype(np.float64)).sum(axis=1).astype(np.float32)

    return out.reshape(B, N_SEQ, C)
